# revision 6
# baseline (speedup 1.0000x reference)
"""LFQ (lookup-free quantization) Trainium2 kernel.

Computes, for x [16, 512, 4096]:
  orig    = (w_in @ x + b_in) transposed to [b, t, 10]   (weight-normed 1x1 conv)
  quant   = sign(orig) in {-1, +1}
  indices = bits(quant) as int32 [b, t]
  out     = w_out @ quant + b_out   [b, 512, t]
  aux     = per_sample_entropy - codebook_entropy + commit_loss (scalar)

Key algebraic fact used: the codebook is the full {-1,+1}^10 hypercube, so the
softmax over 1024 codes factorizes into 10 independent Bernoullis with
p(bit_d=1) = sigmoid(400 * orig_d).  Hence:
  per_sample_entropy = sum_d H_b(sigmoid(400*orig_d))      (binary entropies)
  prob(j)            = prod_d marginal_d(bit_jd)           (exact)
  avg_prob           = mean_t P5(t) (x) Q5(t)  == P5^T @ Q5 / N  (32x32 matmul)
where P5/Q5 are the outer products of the first/last 5 marginals.

Sharding: data-parallel over batch, 2 batches per core on 8 cores. Scalar-loss
partial sums + the 32x32 avg_prob partial matrix are returned per-core and
combined on the host (a few KB).
"""

import sys

for _p in ("/opt/trn_rl_repo",):
    if _p not in sys.path:
        sys.path.insert(0, _p)

import numpy as np
import ml_dtypes

import concourse.bacc as bacc
import concourse.bass as bass
import concourse.mybir as mybir
import concourse.tile as tile
from concourse.bass_utils import run_bass_kernel_spmd

F32 = mybir.dt.float32
BF16 = mybir.dt.bfloat16
I32 = mybir.dt.int32
AF = mybir.ActivationFunctionType
OP = mybir.AluOpType

N_CORES = 8
B, D, T = 16, 512, 4096
CD = 10  # codebook dim
BPC = B // N_CORES          # batches per core
NPOS = BPC * T              # positions per core
GB_POS = 2048               # positions per G-block (one half of a batch's T)
NGB = NPOS // GB_POS        # G-blocks per core
NK = D // 128               # k-tiles over the contraction dim
NM = D // 128               # m-tiles of the output channel dim
NTILE = GB_POS // 128       # 128-position tiles per G-block
NCH = GB_POS // 512         # 512-position chunks per G-block


def _emit(tc, nc, io, ctx):
    x_d, y_d, idx_d = io["x"], io["y"], io["idx"]
    avg_d, hacc_d, cacc_d = io["avg32"], io["hacc"], io["cacc"]

    consts = ctx.enter_context(tc.tile_pool(name="consts", bufs=1))
    xpool = ctx.enter_context(tc.tile_pool(name="xin", bufs=2))
    outp = ctx.enter_context(tc.tile_pool(name="outp", bufs=2))
    work = ctx.enter_context(tc.tile_pool(name="work", bufs=2))
    qp = ctx.enter_context(tc.tile_pool(name="qp", bufs=2))
    idxp = ctx.enter_context(tc.tile_pool(name="idxp", bufs=1))
    accp = ctx.enter_context(tc.tile_pool(name="accp", bufs=1))
    ps1p = ctx.enter_context(tc.tile_pool(name="ps1", bufs=2, space="PSUM"))
    ps2p = ctx.enter_context(tc.tile_pool(name="ps2", bufs=2, space="PSUM"))
    spsp = ctx.enter_context(tc.tile_pool(name="sps", bufs=2, space="PSUM"))
    avgp = ctx.enter_context(tc.tile_pool(name="avgps", bufs=1, space="PSUM"))

    # --- constants (DMA'd once) ---
    w_sb = consts.tile([128, NK, CD], F32)          # w_inT k-tiles
    nc.sync.dma_start(w_sb[:], io["w_inT"][:])
    woh_sb = consts.tile([CD, D], BF16)             # w_outT hi
    nc.sync.dma_start(woh_sb[:], io["w_outT_hi"][:])
    wol_sb = consts.tile([CD, D], BF16)             # w_outT lo
    nc.sync.dma_start(wol_sb[:], io["w_outT_lo"][:])
    bb_sb = consts.tile([128, 160], F32)            # in_b tiled 16x
    nc.sync.dma_start(bb_sb[:], io["b_bcast"][:])
    obt_sb = consts.tile([128, NM], F32)            # out_b m-tiled
    nc.sync.dma_start(obt_sb[:], io["outb_t"][:])
    id_sb = consts.tile([128, 128], F32)            # identity for PE transpose
    nc.sync.dma_start(id_sb[:], io["ident"][:])
    mh_sb = consts.tile([CD, 1], BF16)              # mask/2 column
    nc.sync.dma_start(mh_sb[:], io["maskh"][:])

    # --- persistent accumulators ---
    hacc = accp.tile([128, NGB], F32)
    cacc = accp.tile([128, NGB], F32)
    avgps = avgp.tile([32, 32], F32)
    neg1 = consts.tile([128, 1], F32)
    nc.vector.memset(neg1[:], -1.0)

    for gb in range(NGB):
        b = gb // (T // GB_POS)
        t0 = (gb % (T // GB_POS)) * GB_POS

        # ---- load x G-block: [128, k, 2048] (D = k*128 + p) ----
        xk = xpool.tile([128, NK, GB_POS], F32, tag="xk")
        x_src = x_d[b, :, t0 : t0 + GB_POS].rearrange("(k p) t -> p k t", p=128)
        nc.sync.dma_start(xk[:], x_src)

        # ---- matmul1: orig-partial [128 pos, 10] per pos-tile, pos-major ----
        ps1 = ps1p.tile([128, NTILE * CD], F32, tag="ps1")
        for g in range(NTILE):
            for k in range(NK):
                nc.tensor.matmul(
                    ps1[:, g * CD : (g + 1) * CD],
                    lhsT=xk[:, k, g * 128 : (g + 1) * 128],
                    rhs=w_sb[:, k, :],
                    start=(k == 0),
                    stop=(k == NK - 1),
                )

        # ---- orig = ps1 + b  (pos-major [128, 160]) ----
        orig = work.tile([128, 160], F32, tag="orig")
        nc.vector.tensor_add(orig[:], ps1[:], bb_sb[:])

        # ---- elementwise entropy/commit path ----
        v = work.tile([128, 160], F32, tag="v")      # 400*|orig|
        nc.scalar.activation(v[:], orig[:], AF.Abs, scale=400.0)
        cms = work.tile([128, 160], F32, tag="cms")  # (|orig|-1)^2, keep row-sums
        nc.scalar.activation(
            cms[:], v[:], AF.Square, bias=neg1[:], scale=1.0 / 400.0,
            accum_out=cacc[:, gb : gb + 1],
        )
        t_ = work.tile([128, 160], F32, tag="t_")    # exp(-v)
        nc.scalar.activation(t_[:], v[:], AF.Exp, scale=-1.0)
        w1 = work.tile([128, 160], F32, tag="w1")    # 1 + t
        nc.vector.tensor_scalar_add(w1[:], t_[:], 1.0)
        r = work.tile([128, 160], F32, tag="r")      # 1/(1+t)
        nc.vector.reciprocal(r[:], w1[:])
        lg = work.tile([128, 160], F32, tag="lg")    # ln(1+t)
        nc.scalar.activation(lg[:], w1[:], AF.Ln)
        tr = work.tile([128, 160], F32, tag="tr")    # t/(1+t)
        nc.vector.tensor_mul(tr[:], t_[:], r[:])
        hh = work.tile([128, 160], F32, tag="hh")    # v * t/(1+t)
        nc.vector.tensor_mul(hh[:], v[:], tr[:])
        hsc = work.tile([128, 160], F32, tag="hsc")  # H = ln(1+t) + v*t/(1+t)
        nc.vector.scalar_tensor_tensor(
            hsc[:], hh[:], 0.0, lg[:], op0=OP.add, op1=OP.add,
            accum_out=hacc[:, gb : gb + 1],
        )

        # ---- quantized (+-1) and marginal factors ----
        q32 = work.tile([128, 160], F32, tag="q32")
        nc.scalar.activation(q32[:], orig[:], AF.Sign)
        dd = work.tile([128, 160], F32, tag="dd")    # r - tr = (1-t)/(1+t)
        nc.vector.tensor_sub(dd[:], r[:], tr[:])
        qd = work.tile([128, 160], F32, tag="qd")    # q * (1-t)/(1+t)
        nc.vector.tensor_mul(qd[:], q32[:], dd[:])

        # V[p, d, bit, g]: marginal prob of bit value, = 0.5 +- 0.5*qd
        vt = work.tile([128, CD, 2, NTILE], F32, tag="vt")
        qd_dg = qd[:].rearrange("p (g d) -> p d g", d=CD)
        nc.vector.tensor_scalar(
            vt[:, :, 1, :], qd_dg, 0.5, 0.5, op0=OP.mult, op1=OP.add
        )
        nc.vector.tensor_scalar(
            vt[:, :, 0, :], qd_dg, -0.5, 0.5, op0=OP.mult, op1=OP.add
        )

        # ---- product trees -> P5, Q5 [128, 32, NTILE] ----
        def tree(d0, tag):
            lvl = vt[:, d0, :, :]  # [128, 2, NTILE]
            n = 2
            outs = []
            for i, d in enumerate(range(d0 + 1, d0 + 5)):
                nxt = work.tile([128, n * 2, NTILE], F32, tag=f"{tag}{i}")
                nc.vector.tensor_tensor(
                    nxt[:].rearrange("p (j c) g -> p j c g", c=2),
                    lvl[:, :, None, :].broadcast_to([128, n, 2, NTILE]),
                    vt[:, d, None, :, :].broadcast_to([128, n, 2, NTILE]),
                    op=OP.mult,
                )
                lvl = nxt[:]
                n *= 2
                outs.append(nxt)
            return outs[-1]

        p5 = tree(0, "p5t")
        q5 = tree(5, "q5t")

        # ---- avg_prob partial: accumulate P5^T @ Q5 over all pos-tiles ----
        for g in range(NTILE):
            nc.tensor.matmul(
                avgps[:],
                lhsT=p5[:, :, g],
                rhs=q5[:, :, g],
                start=(gb == 0 and g == 0),
                stop=(gb == NGB - 1 and g == NTILE - 1),
            )

        # ---- transpose quantized to [10, pos] (bf16) ----
        qb = qp.tile([CD, GB_POS], BF16, tag="qb")
        for c in range(NCH):
            qt_ps = spsp.tile([CD, 512], F32, tag="sps")
            for u in range(4):
                g = c * 4 + u
                nc.tensor.transpose(
                    qt_ps[:, u * 128 : (u + 1) * 128],
                    q32[:, g * CD : (g + 1) * CD],
                    id_sb[:],
                )
            nc.vector.tensor_copy(qb[:, c * 512 : (c + 1) * 512], qt_ps[:])

        # ---- indices: 0.5*sum(q*mask) + 511.5, via K=10 matmul ----
        idx_t = idxp.tile([1, GB_POS], I32, tag="idx")
        for c in range(NCH):
            ip = spsp.tile([1, 512], F32, tag="sps")
            nc.tensor.matmul(
                ip[:], lhsT=mh_sb[:], rhs=qb[:, c * 512 : (c + 1) * 512],
                start=True, stop=True,
            )
            nc.any.tensor_scalar_add(idx_t[:, c * 512 : (c + 1) * 512], ip[:], 511.5)
        nc.sync.dma_start(idx_d[b, t0 : t0 + GB_POS][None, :], idx_t[:])

        # ---- matmul2: out = w_out @ quant + b_out ----
        out_t = outp.tile([128, NM, GB_POS], F32, tag="outt")
        for m in range(NM):
            for c in range(NCH):
                ps2 = ps2p.tile([128, 512], F32, tag="ps2")
                qs = qb[:, c * 512 : (c + 1) * 512]
                nc.tensor.matmul(
                    ps2[:], lhsT=woh_sb[:, m * 128 : (m + 1) * 128], rhs=qs,
                    start=True, stop=False,
                )
                nc.tensor.matmul(
                    ps2[:], lhsT=wol_sb[:, m * 128 : (m + 1) * 128], rhs=qs,
                    start=False, stop=True,
                )
                nc.any.tensor_scalar_add(
                    out_t[:, m, c * 512 : (c + 1) * 512], ps2[:],
                    obt_sb[:, m : m + 1],
                )
        y_dst = y_d[b, :, t0 : t0 + GB_POS].rearrange("(m p) t -> p m t", p=128)
        nc.sync.dma_start(y_dst, out_t[:])

    # ---- final small outputs ----
    av_sb = accp.tile([32, 32], F32)
    nc.any.tensor_copy(av_sb[:], avgps[:])
    nc.sync.dma_start(avg_d[:], av_sb[:])
    nc.sync.dma_start(hacc_d[:], hacc[:])
    nc.sync.dma_start(cacc_d[:], cacc[:])


_CACHE = {}


def _build():
    if "nc" in _CACHE:
        return _CACHE["nc"]
    nc = bacc.Bacc(
        "TRN2", target_bir_lowering=False, debug=False,
        enable_asserts=True, num_devices=N_CORES,
    )
    io = {}
    io["x"] = nc.dram_tensor("x", [BPC, D, T], F32, kind="ExternalInput").ap()
    io["w_inT"] = nc.dram_tensor("w_inT", [128, NK, CD], F32, kind="ExternalInput").ap()
    io["w_outT_hi"] = nc.dram_tensor("w_outT_hi", [CD, D], BF16, kind="ExternalInput").ap()
    io["w_outT_lo"] = nc.dram_tensor("w_outT_lo", [CD, D], BF16, kind="ExternalInput").ap()
    io["b_bcast"] = nc.dram_tensor("b_bcast", [128, 160], F32, kind="ExternalInput").ap()
    io["outb_t"] = nc.dram_tensor("outb_t", [128, NM], F32, kind="ExternalInput").ap()
    io["ident"] = nc.dram_tensor("ident", [128, 128], F32, kind="ExternalInput").ap()
    io["maskh"] = nc.dram_tensor("maskh", [CD, 1], BF16, kind="ExternalInput").ap()
    io["y"] = nc.dram_tensor("y", [BPC, D, T], F32, kind="ExternalOutput").ap()
    io["idx"] = nc.dram_tensor("idx", [BPC, T], I32, kind="ExternalOutput").ap()
    io["avg32"] = nc.dram_tensor("avg32", [32, 32], F32, kind="ExternalOutput").ap()
    io["hacc"] = nc.dram_tensor("hacc", [128, NGB], F32, kind="ExternalOutput").ap()
    io["cacc"] = nc.dram_tensor("cacc", [128, NGB], F32, kind="ExternalOutput").ap()

    from contextlib import ExitStack

    with tile.TileContext(nc) as tc:
        with ExitStack() as ctx:
            _emit(tc, nc, io, ctx)
    nc.compile()
    _CACHE["nc"] = nc
    return nc


def _host_prep(in_v, in_g, in_b, out_v, out_g, out_b):
    """Weight-norm + layout prep (tiny, fp32 to match the reference)."""
    in_v = in_v.astype(np.float32)
    out_v = out_v.astype(np.float32)
    w_in = in_g[:, None].astype(np.float32) * in_v / np.linalg.norm(
        in_v, axis=1, keepdims=True
    ).astype(np.float32)                                   # [10, 512]
    w_out = out_g[:, None].astype(np.float32) * out_v / np.linalg.norm(
        out_v, axis=1, keepdims=True
    ).astype(np.float32)                                   # [512, 10]

    w_inT = np.ascontiguousarray(
        w_in.T.reshape(NK, 128, CD).transpose(1, 0, 2)
    )                                                      # [128, k, 10]
    w_outT = np.ascontiguousarray(w_out.T)                 # [10, 512]
    w_outT_hi = w_outT.astype(ml_dtypes.bfloat16)
    w_outT_lo = (w_outT - w_outT_hi.astype(np.float32)).astype(ml_dtypes.bfloat16)

    b_bcast = np.tile(in_b.astype(np.float32)[None, :], (128, 16))  # [128,160]
    outb_t = np.ascontiguousarray(
        out_b.astype(np.float32).reshape(NM, 128).T
    )                                                      # [128, m]
    ident = np.eye(128, dtype=np.float32)
    maskh = (2.0 ** np.arange(CD - 1, -1, -1) / 2.0).reshape(CD, 1).astype(
        ml_dtypes.bfloat16
    )
    return dict(
        w_inT=w_inT, w_outT_hi=w_outT_hi, w_outT_lo=w_outT_lo,
        b_bcast=b_bcast, outb_t=outb_t, ident=ident, maskh=maskh,
    )


def run_cores(x, consts):
    """Run the SPMD kernel; returns the per-core result dicts."""
    nc = _build()
    in_maps = []
    for c in range(N_CORES):
        m = dict(consts)
        m["x"] = np.ascontiguousarray(x[c * BPC : (c + 1) * BPC])
        in_maps.append(m)
    res = run_bass_kernel_spmd(nc, in_maps, core_ids=list(range(N_CORES)))
    return res.results


def kernel(x, in_v, in_g, in_b, out_v, out_g, out_b):
    x = np.asarray(x, dtype=np.float32)
    consts = _host_prep(
        np.asarray(in_v), np.asarray(in_g), np.asarray(in_b),
        np.asarray(out_v), np.asarray(out_g), np.asarray(out_b),
    )
    results = run_cores(x, consts)

    out = np.concatenate([r["y"] for r in results], axis=0)
    indices = np.concatenate([r["idx"] for r in results], axis=0).astype(np.int32)

    avg = np.zeros((32, 32), np.float64)
    hsum = 0.0
    csum = 0.0
    for r in results:
        avg += r["avg32"].astype(np.float64)
        hsum += r["hacc"].astype(np.float64).sum()
        csum += r["cacc"].astype(np.float64).sum()

    n_pos = B * T
    avg_prob = (avg / n_pos).reshape(1024)
    codebook_entropy = float(
        -(avg_prob * np.log(np.maximum(avg_prob, 1e-5))).sum()
    )
    per_sample_entropy = hsum / n_pos
    commit = csum / (n_pos * CD)
    aux = np.float32(per_sample_entropy - codebook_entropy + commit)
    return out, indices, aux


# revision 12
# speedup vs baseline: 21.5671x; 21.5671x over previous
"""LFQ (lookup-free quantization) Trainium2 kernel.

Computes, for x [16, 512, 4096]:
  orig    = (w_in @ x + b_in) transposed to [b, t, 10]   (weight-normed 1x1 conv)
  quant   = sign(orig) in {-1, +1}
  indices = bits(quant) as int32 [b, t]
  out     = w_out @ quant + b_out   [b, 512, t]
  aux     = per_sample_entropy - codebook_entropy + commit_loss (scalar)

Key algebraic fact used: the codebook is the full {-1,+1}^10 hypercube, so the
softmax over 1024 codes factorizes into 10 independent Bernoullis with
p(bit_d=1) = sigmoid(400 * orig_d).  Hence:
  per_sample_entropy = sum_d H_b(sigmoid(400*orig_d))      (binary entropies)
  prob(j)            = prod_d marginal_d(bit_jd)           (exact)
  avg_prob           = mean_t P5(t) (x) Q5(t)  == P5^T @ Q5 / N  (32x32 matmul)
where P5/Q5 are the outer products of the first/last 5 marginals.

Sharding: data-parallel over batch, 2 batches per core on 8 cores. Scalar-loss
partial sums + the 32x32 avg_prob partial matrix are returned per-core and
combined on the host (a few KB).
"""

import sys

for _p in ("/opt/trn_rl_repo",):
    if _p not in sys.path:
        sys.path.insert(0, _p)

import numpy as np
import ml_dtypes

import concourse.bacc as bacc
import concourse.bass as bass
import concourse.mybir as mybir
import concourse.tile as tile
from concourse.bass_utils import run_bass_kernel_spmd

F32 = mybir.dt.float32
BF16 = mybir.dt.bfloat16
I32 = mybir.dt.int32
AF = mybir.ActivationFunctionType
OP = mybir.AluOpType

N_CORES = 8
B, D, T = 16, 512, 4096
CD = 10  # codebook dim
BPC = B // N_CORES          # batches per core
NPOS = BPC * T              # positions per core
GB_POS = 2048               # positions per G-block (one half of a batch's T)
NGB = NPOS // GB_POS        # G-blocks per core
NK = D // 128               # k-tiles over the contraction dim
NM = D // 128               # m-tiles of the output channel dim
NTILE = GB_POS // 128       # 128-position tiles per G-block
NCH = GB_POS // 512         # 512-position chunks per G-block


def _emit(tc, nc, io, ctx):
    x_d, y_d, idx_d = io["x"], io["y"], io["idx"]
    avg_d, hacc_d, cacc_d = io["avg32"], io["hacc"], io["cacc"]

    consts = ctx.enter_context(tc.tile_pool(name="consts", bufs=1))
    xpool = ctx.enter_context(tc.tile_pool(name="xin", bufs=2))
    outp = ctx.enter_context(tc.tile_pool(name="outp", bufs=2))
    work = ctx.enter_context(tc.tile_pool(name="work", bufs=2))
    qp = ctx.enter_context(tc.tile_pool(name="qp", bufs=2))
    idxp = ctx.enter_context(tc.tile_pool(name="idxp", bufs=1))
    accp = ctx.enter_context(tc.tile_pool(name="accp", bufs=1))
    ps1p = ctx.enter_context(tc.tile_pool(name="ps1", bufs=2, space="PSUM"))
    ps2p = ctx.enter_context(tc.tile_pool(name="ps2", bufs=2, space="PSUM"))
    spsp = ctx.enter_context(tc.tile_pool(name="sps", bufs=2, space="PSUM"))
    avgp = ctx.enter_context(tc.tile_pool(name="avgps", bufs=1, space="PSUM"))

    # --- constants (DMA'd once) ---
    w_sb = consts.tile([128, NK, CD], F32)          # w_inT k-tiles
    nc.sync.dma_start(w_sb[:], io["w_inT"][:])
    woh_sb = consts.tile([CD, D], BF16)             # w_outT hi
    nc.sync.dma_start(woh_sb[:], io["w_outT_hi"][:])
    wol_sb = consts.tile([CD, D], BF16)             # w_outT lo
    nc.sync.dma_start(wol_sb[:], io["w_outT_lo"][:])
    bb_sb = consts.tile([128, 160], F32)            # in_b tiled 16x
    nc.sync.dma_start(bb_sb[:], io["b_bcast"][:])
    obt_sb = consts.tile([128, NM], F32)            # out_b m-tiled
    nc.sync.dma_start(obt_sb[:], io["outb_t"][:])
    id_sb = consts.tile([128, 128], F32)            # identity for PE transpose
    nc.sync.dma_start(id_sb[:], io["ident"][:])
    mh_sb = consts.tile([CD, 1], BF16)              # mask/2 column
    nc.sync.dma_start(mh_sb[:], io["maskh"][:])

    # --- persistent accumulators ---
    hacc = accp.tile([128, NGB], F32)
    cacc = accp.tile([128, NGB], F32)
    avgps = avgp.tile([32, 32], F32)
    neg1 = consts.tile([128, 1], F32)
    nc.vector.memset(neg1[:], -1.0)

    for gb in range(NGB):
        b = gb // (T // GB_POS)
        t0 = (gb % (T // GB_POS)) * GB_POS

        # ---- load x G-block: [128, k, 2048] (D = k*128 + p) ----
        xk = xpool.tile([128, NK, GB_POS], F32, tag="xk")
        x_src = x_d[b, :, t0 : t0 + GB_POS].rearrange("(k p) t -> p k t", p=128)
        nc.sync.dma_start(xk[:], x_src)

        # ---- matmul1: orig-partial [128 pos, 10] per pos-tile, pos-major ----
        ps1 = ps1p.tile([128, NTILE * CD], F32, tag="ps1")
        for g in range(NTILE):
            for k in range(NK):
                nc.tensor.matmul(
                    ps1[:, g * CD : (g + 1) * CD],
                    lhsT=xk[:, k, g * 128 : (g + 1) * 128],
                    rhs=w_sb[:, k, :],
                    start=(k == 0),
                    stop=(k == NK - 1),
                )

        # ---- orig = ps1 + b  (pos-major [128, 160]) ----
        orig = work.tile([128, 160], F32, tag="orig")
        nc.vector.tensor_add(orig[:], ps1[:], bb_sb[:])

        # ---- elementwise entropy/commit path ----
        v = work.tile([128, 160], F32, tag="v")      # 400*|orig|
        nc.scalar.activation(v[:], orig[:], AF.Abs, scale=400.0)
        cms = work.tile([128, 160], F32, tag="cms")  # (|orig|-1)^2, keep row-sums
        nc.scalar.activation(
            cms[:], v[:], AF.Square, bias=neg1[:], scale=1.0 / 400.0,
            accum_out=cacc[:, gb : gb + 1],
        )
        t_ = work.tile([128, 160], F32, tag="t_")    # exp(-v)
        nc.scalar.activation(t_[:], v[:], AF.Exp, scale=-1.0)
        w1 = work.tile([128, 160], F32, tag="w1")    # 1 + t
        nc.vector.tensor_scalar_add(w1[:], t_[:], 1.0)
        r = work.tile([128, 160], F32, tag="r")      # 1/(1+t)
        nc.vector.reciprocal(r[:], w1[:])
        lg = work.tile([128, 160], F32, tag="lg")    # ln(1+t)
        nc.scalar.activation(lg[:], w1[:], AF.Ln)
        tr = work.tile([128, 160], F32, tag="tr")    # t/(1+t)
        nc.vector.tensor_mul(tr[:], t_[:], r[:])
        hh = work.tile([128, 160], F32, tag="hh")    # v * t/(1+t)
        nc.vector.tensor_mul(hh[:], v[:], tr[:])
        hsc = work.tile([128, 160], F32, tag="hsc")  # H = ln(1+t) + v*t/(1+t)
        nc.vector.scalar_tensor_tensor(
            hsc[:], hh[:], 0.0, lg[:], op0=OP.add, op1=OP.add,
            accum_out=hacc[:, gb : gb + 1],
        )

        # ---- quantized (+-1) and marginal factors ----
        q32 = work.tile([128, 160], F32, tag="q32")
        nc.scalar.activation(q32[:], orig[:], AF.Sign)
        dd = work.tile([128, 160], F32, tag="dd")    # r - tr = (1-t)/(1+t)
        nc.vector.tensor_sub(dd[:], r[:], tr[:])
        qd = work.tile([128, 160], F32, tag="qd")    # q * (1-t)/(1+t)
        nc.vector.tensor_mul(qd[:], q32[:], dd[:])

        # V[p, d, bit, g]: marginal prob of bit value, = 0.5 +- 0.5*qd
        vt = work.tile([128, CD, 2, NTILE], F32, tag="vt")
        qd_dg = qd[:].rearrange("p (g d) -> p d g", d=CD)
        nc.vector.tensor_scalar(
            vt[:, :, 1, :], qd_dg, 0.5, 0.5, op0=OP.mult, op1=OP.add
        )
        nc.vector.tensor_scalar(
            vt[:, :, 0, :], qd_dg, -0.5, 0.5, op0=OP.mult, op1=OP.add
        )

        # ---- product trees -> P5, Q5 [128, 32, NTILE] ----
        def tree(d0, tag):
            lvl = vt[:, d0, :, :]  # [128, 2, NTILE]
            n = 2
            outs = []
            for i, d in enumerate(range(d0 + 1, d0 + 5)):
                nxt = work.tile([128, n * 2, NTILE], F32, tag=f"{tag}{i}")
                nc.vector.tensor_tensor(
                    nxt[:].rearrange("p (j c) g -> p j c g", c=2),
                    lvl[:, :, None, :].broadcast_to([128, n, 2, NTILE]),
                    vt[:, d, None, :, :].broadcast_to([128, n, 2, NTILE]),
                    op=OP.mult,
                )
                lvl = nxt[:]
                n *= 2
                outs.append(nxt)
            return outs[-1]

        p5 = tree(0, "p5t")
        q5 = tree(5, "q5t")

        # ---- avg_prob partial: accumulate P5^T @ Q5 over all pos-tiles ----
        for g in range(NTILE):
            nc.tensor.matmul(
                avgps[:],
                lhsT=p5[:, :, g],
                rhs=q5[:, :, g],
                start=(gb == 0 and g == 0),
                stop=(gb == NGB - 1 and g == NTILE - 1),
            )

        # ---- transpose quantized to [10, pos] (bf16) ----
        qb = qp.tile([CD, GB_POS], BF16, tag="qb")
        for c in range(NCH):
            qt_ps = spsp.tile([CD, 512], F32, tag="sps")
            for u in range(4):
                g = c * 4 + u
                nc.tensor.transpose(
                    qt_ps[:, u * 128 : (u + 1) * 128],
                    q32[:, g * CD : (g + 1) * CD],
                    id_sb[:],
                )
            nc.vector.tensor_copy(qb[:, c * 512 : (c + 1) * 512], qt_ps[:])

        # ---- indices: 0.5*sum(q*mask) + 511.5, via K=10 matmul ----
        idx_t = idxp.tile([1, GB_POS], I32, tag="idx")
        for c in range(NCH):
            ip = spsp.tile([1, 512], F32, tag="sps")
            nc.tensor.matmul(
                ip[:], lhsT=mh_sb[:], rhs=qb[:, c * 512 : (c + 1) * 512],
                start=True, stop=True,
            )
            nc.any.tensor_scalar_add(idx_t[:, c * 512 : (c + 1) * 512], ip[:], 511.5)
        nc.sync.dma_start(idx_d[b, t0 : t0 + GB_POS][None, :], idx_t[:])

        # ---- matmul2: out = w_out @ quant + b_out ----
        out_t = outp.tile([128, NM, GB_POS], F32, tag="outt")
        for m in range(NM):
            for c in range(NCH):
                ps2 = ps2p.tile([128, 512], F32, tag="ps2")
                qs = qb[:, c * 512 : (c + 1) * 512]
                nc.tensor.matmul(
                    ps2[:], lhsT=woh_sb[:, m * 128 : (m + 1) * 128], rhs=qs,
                    start=True, stop=False,
                )
                nc.tensor.matmul(
                    ps2[:], lhsT=wol_sb[:, m * 128 : (m + 1) * 128], rhs=qs,
                    start=False, stop=True,
                )
                nc.any.tensor_scalar_add(
                    out_t[:, m, c * 512 : (c + 1) * 512], ps2[:],
                    obt_sb[:, m : m + 1],
                )
        y_dst = y_d[b, :, t0 : t0 + GB_POS].rearrange("(m p) t -> p m t", p=128)
        nc.sync.dma_start(y_dst, out_t[:])

    # ---- final small outputs ----
    av_sb = accp.tile([32, 32], F32)
    nc.any.tensor_copy(av_sb[:], avgps[:])
    nc.sync.dma_start(avg_d[:], av_sb[:])
    nc.sync.dma_start(hacc_d[:], hacc[:])
    nc.sync.dma_start(cacc_d[:], cacc[:])


_CACHE = {}


def _build():
    if "nc" in _CACHE:
        return _CACHE["nc"]
    nc = bacc.Bacc(
        "TRN2", target_bir_lowering=False, debug=False,
        enable_asserts=True, num_devices=N_CORES,
    )
    io = {}
    io["x"] = nc.dram_tensor("x", [BPC, D, T], F32, kind="ExternalInput").ap()
    io["w_inT"] = nc.dram_tensor("w_inT", [128, NK, CD], F32, kind="ExternalInput").ap()
    io["w_outT_hi"] = nc.dram_tensor("w_outT_hi", [CD, D], BF16, kind="ExternalInput").ap()
    io["w_outT_lo"] = nc.dram_tensor("w_outT_lo", [CD, D], BF16, kind="ExternalInput").ap()
    io["b_bcast"] = nc.dram_tensor("b_bcast", [128, 160], F32, kind="ExternalInput").ap()
    io["outb_t"] = nc.dram_tensor("outb_t", [128, NM], F32, kind="ExternalInput").ap()
    io["ident"] = nc.dram_tensor("ident", [128, 128], F32, kind="ExternalInput").ap()
    io["maskh"] = nc.dram_tensor("maskh", [CD, 1], BF16, kind="ExternalInput").ap()
    io["y"] = nc.dram_tensor("y", [BPC, D, T], F32, kind="ExternalOutput").ap()
    io["idx"] = nc.dram_tensor("idx", [BPC, T], I32, kind="ExternalOutput").ap()
    io["avg32"] = nc.dram_tensor("avg32", [32, 32], F32, kind="ExternalOutput").ap()
    io["hacc"] = nc.dram_tensor("hacc", [128, NGB], F32, kind="ExternalOutput").ap()
    io["cacc"] = nc.dram_tensor("cacc", [128, NGB], F32, kind="ExternalOutput").ap()

    from contextlib import ExitStack

    with tile.TileContext(nc) as tc:
        with ExitStack() as ctx:
            _emit(tc, nc, io, ctx)
    nc.compile()
    _CACHE["nc"] = nc
    return nc


def _host_prep(in_v, in_g, in_b, out_v, out_g, out_b):
    """Weight-norm + layout prep (tiny, fp32 to match the reference)."""
    in_v = in_v.astype(np.float32)
    out_v = out_v.astype(np.float32)
    w_in = in_g[:, None].astype(np.float32) * in_v / np.linalg.norm(
        in_v, axis=1, keepdims=True
    ).astype(np.float32)                                   # [10, 512]
    w_out = out_g[:, None].astype(np.float32) * out_v / np.linalg.norm(
        out_v, axis=1, keepdims=True
    ).astype(np.float32)                                   # [512, 10]

    w_inT = np.ascontiguousarray(
        w_in.T.reshape(NK, 128, CD).transpose(1, 0, 2)
    )                                                      # [128, k, 10]
    w_outT = np.ascontiguousarray(w_out.T)                 # [10, 512]
    w_outT_hi = w_outT.astype(ml_dtypes.bfloat16)
    w_outT_lo = (w_outT - w_outT_hi.astype(np.float32)).astype(ml_dtypes.bfloat16)

    b_bcast = np.tile(in_b.astype(np.float32)[None, :], (128, 16))  # [128,160]
    outb_t = np.ascontiguousarray(
        out_b.astype(np.float32).reshape(NM, 128).T
    )                                                      # [128, m]
    ident = np.eye(128, dtype=np.float32)
    maskh = (2.0 ** np.arange(CD - 1, -1, -1) / 2.0).reshape(CD, 1).astype(
        ml_dtypes.bfloat16
    )
    return dict(
        w_inT=w_inT, w_outT_hi=w_outT_hi, w_outT_lo=w_outT_lo,
        b_bcast=b_bcast, outb_t=outb_t, ident=ident, maskh=maskh,
    )


def _get_executor():
    """Build (once) a cached jitted shard_map executor for the 8-core kernel.

    Mirrors concourse.bass2jax.run_bass_via_pjrt but keeps the jitted callable
    alive so repeat calls skip retracing/recompile, and creates the zero
    output buffers on-device instead of shipping them from the host.
    """
    if "exec" in _CACHE:
        return _CACHE["exec"]
    import jax
    from jax.sharding import Mesh, PartitionSpec
    from jax.experimental.shard_map import shard_map
    import concourse.mybir as mb
    from concourse import bass2jax

    nc = _build()
    bass2jax.install_neuronx_cc_hook()

    part_name = nc.partition_id_tensor.name if nc.partition_id_tensor else None
    in_names, out_names, out_avals = [], [], []
    for alloc in nc.m.functions[0].allocations:
        if not isinstance(alloc, mb.MemoryLocationSet):
            continue
        name = alloc.memorylocations[0].name
        if alloc.kind == "ExternalInput":
            if name != part_name:
                in_names.append(name)
        elif alloc.kind == "ExternalOutput":
            out_names.append(name)
            out_avals.append(
                jax.core.ShapedArray(tuple(alloc.tensor_shape), mb.dt.np(alloc.dtype))
            )

    bind_in_names = in_names + out_names
    if part_name is not None:
        bind_in_names = bind_in_names + [part_name]

    def _body(*args):
        operands = list(args)
        if part_name is not None:
            operands.append(bass2jax.partition_id_tensor())
        outs = bass2jax._bass_exec_p.bind(
            *operands,
            out_avals=tuple(out_avals),
            in_names=tuple(bind_in_names),
            out_names=tuple(out_names),
            lowering_input_output_aliases=(),
            sim_require_finite=True,
            sim_require_nnan=True,
            nc=nc,
        )
        return tuple(outs)

    devices = jax.devices()[:N_CORES]
    mesh = Mesh(np.asarray(devices), ("core",))
    n_args = len(in_names) + len(out_avals)
    in_specs = (PartitionSpec("core"),) * n_args
    out_specs = (PartitionSpec("core"),) * len(out_names)
    fn = jax.jit(
        shard_map(_body, mesh=mesh, in_specs=in_specs, out_specs=out_specs,
                  check_rep=False)
    )
    _CACHE["exec"] = (fn, in_names, out_names, out_avals, mesh)
    return _CACHE["exec"]


def device_inputs(x, consts):
    """Concatenated per-core inputs + zero output-init buffers (numpy)."""
    fn, in_names, out_names, out_avals, mesh = _get_executor()
    per_core = []
    for c in range(N_CORES):
        m = dict(consts)
        m["x"] = np.ascontiguousarray(x[c * BPC : (c + 1) * BPC])
        per_core.append(m)
    arrs = [
        np.concatenate([per_core[c][n] for c in range(N_CORES)], axis=0)
        for n in in_names
    ]
    arrs += [
        np.zeros((N_CORES * a.shape[0], *a.shape[1:]), a.dtype) for a in out_avals
    ]
    return arrs


def run_cores(x, consts, concat_in=None):
    """Run the SPMD kernel; returns the per-core result dicts."""
    fn, in_names, out_names, out_avals, mesh = _get_executor()
    if concat_in is None:
        concat_in = device_inputs(x, consts)
    out_arrs = fn(*concat_in)
    results = []
    for c in range(N_CORES):
        results.append(
            {
                n: np.asarray(out_arrs[i]).reshape(
                    N_CORES, *out_avals[i].shape
                )[c]
                for i, n in enumerate(out_names)
            }
        )
    return results


def run_device_only(concat_in):
    """Execute without fetching outputs to host (for timing)."""
    fn = _get_executor()[0]
    outs = fn(*concat_in)
    for o in outs:
        o.block_until_ready()
    return outs


def kernel(x, in_v, in_g, in_b, out_v, out_g, out_b):
    x = np.asarray(x, dtype=np.float32)
    consts = _host_prep(
        np.asarray(in_v), np.asarray(in_g), np.asarray(in_b),
        np.asarray(out_v), np.asarray(out_g), np.asarray(out_b),
    )
    results = run_cores(x, consts)

    out = np.concatenate([r["y"] for r in results], axis=0)
    indices = np.concatenate([r["idx"] for r in results], axis=0).astype(np.int32)

    avg = np.zeros((32, 32), np.float64)
    hsum = 0.0
    csum = 0.0
    for r in results:
        avg += r["avg32"].astype(np.float64)
        hsum += r["hacc"].astype(np.float64).sum()
        csum += r["cacc"].astype(np.float64).sum()

    n_pos = B * T
    avg_prob = (avg / n_pos).reshape(1024)
    codebook_entropy = float(
        -(avg_prob * np.log(np.maximum(avg_prob, 1e-5))).sum()
    )
    per_sample_entropy = hsum / n_pos
    commit = csum / (n_pos * CD)
    aux = np.float32(per_sample_entropy - codebook_entropy + commit)
    return out, indices, aux
